# revision 39
# baseline (speedup 1.0000x reference)
"""Trainium2 Bass kernel for LUT-based int8-quantized 3x3 conv (ApproxTorch baseline).

Problem: y = conv2d(quant(x), quant(w)) summed via a 256x256 LUT of int8
products, rescaled by (T_f/127)*(T_w/127) + bias, where T_f/T_w are EMA
thresholds updated with the *global* absmax of x / w before the conv.

Key observation: the LUT staged by setup_inputs() is the exact signed-product
table lut[a+128, b+128] = a*b, so the LUT-gather-sum is mathematically an
integer matmul; int8 values in [-128,127] are exactly representable in bf16 and
products accumulate exactly in fp32 PSUM (|sum| < 2^24), so the PE array
computes the reference bit-exactly. We verify the product-table property on the
host and refuse to run otherwise.

Sharding: data-parallel over batch (B=8 -> 1 image/core, 8 cores). Weights and
bias are replicated. The global absmax of x is computed redundantly on every
core from a full replica of x ([128, 3136] layout): +1.6 MB of DMA (~4.5 us)
beats the ~20 us mesh-AllReduce latency floor for a tiny collective and leaves
the kernel with zero cross-core dependencies (immune to core start skew).

PE packing: the 9 conv taps are paired along kh so the contraction uses all
128 PE rows. x is DMAed twice into a [128, 784] tile; the quantized copy in
partitions 64:127 is written one padded row UP, so a single [128, 14, 28]
moving AP feeds tap (0,kw) from the top half and tap (1,kw) from the bottom
half of the same window. 3 pair-groups (K=128) + 3 kh=2 singles (K=64), each
split into two 392-column PSUM banks -> 12 matmuls, 4704 streamed columns
(vs 18 / 7056 unpaired).

Per-core pipeline:
  1. DMA xall [128,3136] (8 chunks, absmax partial-reduced as chunks land),
     x twice into [128,784], wpair [128,384], bias, consts.
  2. absmax -> PE-transpose -> row scalars; EMA thresholds; qscale = 127*(1/T)
     (bit-exact DVE reciprocal), s = T*(1/127), ss = s_x*s_w; broadcast via
     K=1 matmul with a ones row.
  3. Quantize (ACT Copy(v*qs+MAGIC) -> DVE (t-MAGIC) max -128 -> min 127 to
     bf16; MAGIC = 1.5*2^23 gives IEEE round-to-nearest-even = jnp.round).
  4. 6 matmul groups x 2 PSUM halves, accumulate.
  5. out = psum*ss + bias (DVE), DMA out.
"""

import os
import sys

import numpy as np

for _p in ("/opt/trn_rl_repo", "/root/.axon_site", "/root/.axon_site/_ro/trn_rl_repo",
           "/root/.axon_site/_ro/pypackages"):
    if os.path.isdir(_p) and _p not in sys.path:
        sys.path.append(_p)

from concourse import bacc, bass, bass_isa, mybir, tile  # noqa: E402
from concourse.bass_utils import run_bass_kernel_spmd  # noqa: E402

F32 = mybir.dt.float32
BF16 = mybir.dt.bfloat16
AX = mybir.AxisListType
OP = mybir.AluOpType
ACTF = mybir.ActivationFunctionType

N_CORES = 8
CIN = 64
COUT = 64
K = 3
H = W = 28
P = H * W            # 784 pixels
PH = P // 2          # 392 per PSUM bank (14 output rows)
PAD = 30             # padded spatial edge
XD_F = PAD * PAD + PAD + 3 + 31  # x input row: padded image + pad + bias/consts
XALL_P = 128
XALL_F = ((N_CORES - 1) * CIN * P) // XALL_P  # 2744: the 7 other images
NG = 6               # matmul groups: 3 kh-pairs + 3 kh=2 singles
MAGIC = 12582912.0   # 1.5 * 2**23: fp32 add/sub round-to-nearest-even trick

EMA_MUL = 0.05
T_CONSTS = (2.85, 0.285)  # 0.95*T_FEATURE, 0.95*T_WEIGHT as fp32
INV127 = float(np.float32(1.0) / np.float32(127.0))
INV127SQ = float(np.float32(INV127) * np.float32(INV127))


def _build():
    nc = bacc.Bacc(
        "TRN2",
        target_bir_lowering=False,
        debug=False,
        enable_asserts=True,
        num_devices=N_CORES,
    )
    xall_d = nc.dram_tensor("xall", [XALL_P, XALL_F + NG * COUT], F32,
                            kind="ExternalInput")
    x_d = nc.dram_tensor("x", [CIN, XD_F], F32, kind="ExternalInput")
    out_d = nc.dram_tensor("out", [COUT, P], F32, kind="ExternalOutput")

    with tile.TileContext(nc) as tc:
        with (
            tc.tile_pool(name="sbuf", bufs=1) as pool,
            tc.tile_pool(name="psum", bufs=1, space="PSUM") as psum,
        ):
            # ---- loads. Each dma_start spans all 128 partitions so its
            # descriptors fan out across all 16 SDMA engines (8 partitions
            # per engine). dma_start issue (DIRECT2D) costs ~0.6 us serial
            # per HWDGE sequencer, so spread issues across sync + scalar.
            # xall in 4 column-quarters so the absmax partial-reduce of
            # quarter q starts as soon as it lands.
            # x arrives host-padded [64, 964]: [30,30] pad-1 image, 30 zeros,
            # then bias in col 930 and (2.85, 0.285) columns at 931:933.
            # Bottom half reads at +30 (one padded row up) so kh-pair matmul
            # windows hit tap kh+1 in partitions 64:128.
            # xall holds the OTHER 7 images with wpair appended as columns
            # 2744:3128 (rides along with >=4 KB descriptors, one less issue);
            # own image absmax comes from x_sb.
            xall = pool.tile([XALL_P, XALL_F + NG * COUT], F32)
            bnds = (0, 1024, 2048, XALL_F)
            nc.sync.dma_start(out=xall[:, 0:1024], in_=xall_d[:, 0:1024])
            nc.sync.dma_start(out=xall[:, 1024:2048],
                              in_=xall_d[:, 1024:2048])
            nc.scalar.dma_start(out=xall[:, 2048:XALL_F + NG * COUT],
                                in_=xall_d[:, 2048:XALL_F + NG * COUT])
            w_sb = xall[:, XALL_F:XALL_F + NG * COUT]
            x_sb = pool.tile([2 * CIN, XD_F - PAD], F32)
            nc.scalar.dma_start(out=x_sb[0:CIN, :], in_=x_d[:, 0:XD_F - PAD])
            nc.scalar.dma_start(out=x_sb[CIN:2 * CIN, :], in_=x_d[:, PAD:XD_F])
            bias_sb = x_sb[0:COUT, PAD * PAD + PAD:PAD * PAD + PAD + 1]
            crow2 = x_sb[:, PAD * PAD + PAD + 1:PAD * PAD + PAD + 3]

            # ---- absmax partials: w, own image (pads reduce to 0, and the
            # bias/consts tail columns are excluded), one per xall quarter
            pack = pool.tile([XALL_P, 2], F32)
            nc.vector.tensor_reduce(out=pack[:, 1:2], in_=w_sb, axis=AX.X,
                                    op=OP.max, apply_absolute_value=True)
            parts = pool.tile([XALL_P, len(bnds)], F32)
            nc.vector.tensor_reduce(out=parts[:, len(bnds) - 1:len(bnds)],
                                    in_=x_sb[:, 0:PAD * PAD], axis=AX.X,
                                    op=OP.max, apply_absolute_value=True)
            for h in range(len(bnds) - 1):
                nc.vector.tensor_reduce(out=parts[:, h:h + 1],
                                        in_=xall[:, bnds[h]:bnds[h + 1]],
                                        axis=AX.X, op=OP.max,
                                        apply_absolute_value=True)
            nc.vector.tensor_reduce(out=pack[:, 0:1], in_=parts[:], axis=AX.X,
                                    op=OP.max)

            # ---- cross-partition max, already broadcast to all partitions
            gmax = pool.tile([XALL_P, 2], F32)
            nc.gpsimd.partition_all_reduce(gmax[:], pack[:], channels=XALL_P,
                                           reduce_op=bass_isa.ReduceOp.max)

            # ---- scalar math, computed redundantly on all 128 partitions
            # T = gmax*0.05 + (2.85, 0.285); two ops to force fp32 rounding
            t1 = pool.tile([XALL_P, 2], F32)
            nc.vector.tensor_scalar(out=t1[:], in0=gmax[:], scalar1=EMA_MUL,
                                    scalar2=None, op0=OP.mult)
            trow = pool.tile([XALL_P, 2], F32)
            nc.vector.tensor_tensor(out=trow[:], in0=t1[:], in1=crow2,
                                    op=OP.add)
            # qscale = 127*(1/T); s = T*(1/127); ss = s_x*s_w (matches the
            # reference's fp32 rounding bit-for-bit)
            recip = pool.tile([XALL_P, 2], F32)
            nc.vector.reciprocal(recip[:], trow[:])
            scales = pool.tile([XALL_P, 3], F32)
            nc.vector.tensor_scalar(out=scales[:, 0:2], in0=recip[:],
                                    scalar1=127.0, scalar2=None, op0=OP.mult)
            srow = pool.tile([XALL_P, 2], F32)
            nc.vector.tensor_scalar(out=srow[:], in0=trow[:], scalar1=INV127,
                                    scalar2=None, op0=OP.mult)
            nc.vector.tensor_tensor(out=scales[:, 2:3], in0=srow[:, 0:1],
                                    in1=srow[:, 1:2], op=OP.mult)

            # ---- quantize x in two row-chunks (rows 0:16 feed the ph0
            # matmuls, which then overlap quantization of rows 16:30), and w
            # between them. All ops fully contiguous (padding quantizes to 0).
            # step-2 writes bf16: surviving values are ints in [-128, 256)
            # (exact in bf16); anything larger is clipped to 127 by the min
            # regardless of bf16 rounding. step-3 then runs bf16->bf16 (DVE
            # 4x mode). w's DVE steps split by halves so only groups 0-2
            # gate the first matmul.
            CH = 16 * PAD  # 480: first-chunk columns (rows 0:16)
            WH = 3 * COUT  # 192: weight columns for groups 0-2
            tx = pool.tile([2 * CIN, PAD * PAD], F32)
            rx = pool.tile([2 * CIN, PAD * PAD], BF16)
            qx2f = pool.tile([2 * CIN, PAD * PAD], BF16)
            tw = pool.tile([2 * CIN, NG * COUT], F32)
            rw = pool.tile([2 * CIN, NG * COUT], BF16)
            qw = pool.tile([2 * CIN, NG * COUT], BF16)

            nc.scalar.activation(tx[:, 0:CH], x_sb[:, 0:CH], ACTF.Copy,
                                 bias=MAGIC, scale=scales[:, 0:1])
            nc.scalar.activation(tw[:], w_sb, ACTF.Copy,
                                 bias=MAGIC, scale=scales[:, 1:2])
            nc.scalar.activation(tx[:, CH:PAD * PAD], x_sb[:, CH:PAD * PAD],
                                 ACTF.Copy, bias=MAGIC, scale=scales[:, 0:1])
            nc.vector.tensor_scalar(out=rx[:, 0:CH], in0=tx[:, 0:CH],
                                    scalar1=MAGIC, scalar2=-128.0,
                                    op0=OP.subtract, op1=OP.max)
            nc.vector.tensor_scalar(out=qx2f[:, 0:CH], in0=rx[:, 0:CH],
                                    scalar1=127.0, scalar2=None, op0=OP.min)
            for lo, hi in ((0, WH), (WH, NG * COUT)):
                nc.vector.tensor_scalar(out=rw[:, lo:hi], in0=tw[:, lo:hi],
                                        scalar1=MAGIC, scalar2=-128.0,
                                        op0=OP.subtract, op1=OP.max)
                nc.vector.tensor_scalar(out=qw[:, lo:hi], in0=rw[:, lo:hi],
                                        scalar1=127.0, scalar2=None,
                                        op0=OP.min)
            nc.vector.tensor_scalar(out=rx[:, CH:PAD * PAD],
                                    in0=tx[:, CH:PAD * PAD],
                                    scalar1=MAGIC, scalar2=-128.0,
                                    op0=OP.subtract, op1=OP.max)
            nc.vector.tensor_scalar(out=qx2f[:, CH:PAD * PAD],
                                    in0=rx[:, CH:PAD * PAD],
                                    scalar1=127.0, scalar2=None, op0=OP.min)
            qx2 = qx2f[:].rearrange("p (h w) -> p h w", h=PAD)

            # ---- conv: 3 kh-pair groups (K=128) + 3 kh=2 singles (K=64);
            # all ph0 first (needs only qx2 rows 0:16), then ph1
            ph0 = psum.tile([COUT, PH], F32)
            ph1 = psum.tile([COUT, PH], F32)
            for half, ph in ((0, ph0), (1, ph1)):
                for g in range(NG):
                    if g < 3:  # taps (0,kw) + (1,kw), kw = g
                        kh, kw, kp = 0, g, 2 * CIN
                    else:      # tap (2,kw), kw = g - 3
                        kh, kw, kp = 2, g - 3, CIN
                    lhsT = qw[0:kp, g * COUT:(g + 1) * COUT]
                    r0 = kh + 14 * half
                    nc.tensor.matmul(
                        ph[:], lhsT, qx2[0:kp, r0:r0 + 14, kw:kw + W],
                        start=(g == 0), stop=(g == NG - 1))

            # ---- epilogue: out = psum*ss + bias
            out_sb = pool.tile([COUT, P], F32)
            nc.vector.tensor_scalar(out=out_sb[:, 0:PH], in0=ph0[:],
                                    scalar1=scales[0:COUT, 2:3],
                                    scalar2=bias_sb,
                                    op0=OP.mult, op1=OP.add)
            nc.sync.dma_start(out=out_d[:, 0:PH], in_=out_sb[:, 0:PH])
            nc.vector.tensor_scalar(out=out_sb[:, PH:P], in0=ph1[:],
                                    scalar1=scales[0:COUT, 2:3],
                                    scalar2=bias_sb,
                                    op0=OP.mult, op1=OP.add)
            nc.sync.dma_start(out=out_d[:, PH:P], in_=out_sb[:, PH:P])

    nc.compile()
    return nc


_NC = None


def _get_nc():
    global _NC
    if _NC is None:
        _NC = _build()
    return _NC


def _prep_in_maps(x, weight, bias):
    x = np.ascontiguousarray(x, dtype=np.float32)
    # per-core xall: the other 7 images reshaped to [128, XALL_F], with
    # wpair appended as trailing columns (built after wpair below)
    # host-padded per-image layout [64, XD_F]: pad-1 image + 30 zeros, then
    # bias in col 930 and (2.85, 0.285) as full columns 931:933
    xpad = np.zeros((N_CORES, CIN, XD_F), dtype=np.float32)
    xpad[:, :, :PAD * PAD].reshape(N_CORES, CIN, PAD, PAD)[
        :, :, 1:1 + H, 1:1 + W] = x.reshape(N_CORES, CIN, H, W)
    xpad[:, :, PAD * PAD + PAD] = np.asarray(bias, dtype=np.float32)[None, :]
    # consts visible at cols 931:933 to BOTH halves (bottom reads at +30)
    for off in (PAD * PAD + PAD + 1, PAD * PAD + 2 * PAD + 1):
        xpad[:, :, off] = T_CONSTS[0]
        xpad[:, :, off + 1] = T_CONSTS[1]
    # wpair [2*Cin, 6*Cout]: groups 0-2 pair taps (0,kw)/(1,kw) across the
    # partition halves; groups 3-5 hold tap (2,kw) in the top half only.
    wpair = np.zeros((2 * CIN, NG * COUT), dtype=np.float32)
    wt = np.transpose(weight, (1, 2, 3, 0))  # [Cin, kh, kw, Cout]
    for g in range(3):
        wpair[0:CIN, g * COUT:(g + 1) * COUT] = wt[:, 0, g, :]
        wpair[CIN:2 * CIN, g * COUT:(g + 1) * COUT] = wt[:, 1, g, :]
        wpair[0:CIN, (3 + g) * COUT:(4 + g) * COUT] = wt[:, 2, g, :]
    xalls = [np.concatenate(
        [np.concatenate([x[:b], x[b + 1:]]).reshape(XALL_P, XALL_F), wpair],
        axis=1) for b in range(N_CORES)]
    in_maps = []
    for b in range(N_CORES):
        in_maps.append({
            "xall": xalls[b],
            "x": xpad[b],
        })
    return in_maps


def _check_lut(lut):
    idx = np.arange(-128, 128, dtype=np.float32)
    expect = np.outer(idx, idx)
    if not np.array_equal(np.asarray(lut, dtype=np.float32), expect):
        raise ValueError(
            "lut is not the exact int8 product table; this kernel's PE-matmul "
            "formulation only applies to the exact-product LUT.")


def kernel(x, weight, bias, lut):
    _check_lut(lut)
    nc = _get_nc()
    in_maps = _prep_in_maps(np.asarray(x), np.asarray(weight), np.asarray(bias))
    res = run_bass_kernel_spmd(nc, in_maps, core_ids=list(range(N_CORES)))
    out = np.empty((N_CORES, COUT, H, W), dtype=np.float32)
    for b in range(N_CORES):
        out[b] = res.results[b]["out"].reshape(COUT, H, W)
    return out
